# revision 19
# baseline (speedup 1.0000x reference)
"""Bass/Tile TRN2 kernel for nn_AttentionANEWraperChannelsFirstWithCache.

Tensor-parallel over heads across 8 NeuronCores:
  - 28 q heads padded to 32 slots (4 per core; odd cores carry 1 zero dummy).
  - core c owns kv head c//2 (each kv head replicated on a core pair).
  - per core: q/k/v projections for own slots, RoPE, in-SBUF cache update
    (K cache transposed to [d, s] via DMA-xbar transpose), attention over the
    full 4096-row cache in [s, l] layout with slots processed in pairs
    (scores/exp at free dim 1024), softmax denominator accumulated on DVE
    with a single fp32 ones-matmul per slot, normalization broadcast on
    GPSIMD.
  - per-slot AllGather of head outputs overlapped with later attention;
    column-parallel o_proj (448 output rows per core) at the end. Host
    concatenates the 8 row shards.

Matmul operands are bf16 (fp32 PSUM accumulation); softmax stats and
normalization stay fp32.
"""

import math
import numpy as np

H, KV, HD, LI = 28, 4, 128, 5
S_MAX, D, L = 4096, 3584, 512
NCORES = 8
SLOTS = 4                  # head slots per core (28 real heads padded to 32)
OSH = D // NCORES          # 448 o_proj output rows per core
NT = D // 128              # 28 contraction tiles over hidden dim
ST = S_MAX // 128          # 32 s-tiles over the cache
SCALE = 1.0 / math.sqrt(HD)


def _head_of(core, slot):
    off = 4 * (core % 2) + slot
    if off >= 7:
        return None                      # dummy slot
    return (core // 2) * 7 + off


# o_proj accumulation order: pair-major (matches the per-pair AllGather),
# then core, then pair half. Slot 3 exists only on even cores.
REAL_JC = [(2 * pi + h, c) for pi in range(2) for c in range(NCORES)
           for h in range(2) if _head_of(c, 2 * pi + h) is not None]


_prog_cache = {}


def _build(cp):
    import concourse.bass as bass
    import concourse.mybir as mybir
    import concourse.tile as tile
    from concourse import bacc
    from contextlib import ExitStack

    f32 = mybir.dt.float32
    bf = mybir.dt.bfloat16
    AF = mybir.ActivationFunctionType
    nc = bacc.Bacc("TRN2", target_bir_lowering=False, debug=False,
                   num_devices=NCORES)

    x_d = nc.dram_tensor("x", [D, L], bf, kind="ExternalInput")
    wqT_d = nc.dram_tensor("wqT", [D, SLOTS * HD], bf, kind="ExternalInput")
    wkT_d = nc.dram_tensor("wkT", [D, HD], bf, kind="ExternalInput")
    wvT_d = nc.dram_tensor("wvT", [D, HD], bf, kind="ExternalInput")
    kc_d = nc.dram_tensor("kcache", [S_MAX, HD], bf, kind="ExternalInput")
    vc_d = nc.dram_tensor("vcache", [S_MAX, HD], bf, kind="ExternalInput")
    trig_d = nc.dram_tensor("trig", [HD, 4, L], f32, kind="ExternalInput")
    bias_d = nc.dram_tensor("biases", [HD, 6], f32, kind="ExternalInput")
    idrot_d = nc.dram_tensor("idrot", [HD, 2, HD], bf, kind="ExternalInput")
    woT_d = nc.dram_tensor("woT", [H * HD, OSH], bf, kind="ExternalInput")
    out_d = nc.dram_tensor("out", [OSH, L], f32, kind="ExternalOutput")

    wt0 = cp // 128                      # first window s-tile
    wset = set(range(wt0, wt0 + L // 128))
    # contiguous cache s-tile ranges outside the update window
    cr = []
    start = None
    for st in range(ST + 1):
        if st < ST and st not in wset:
            if start is None:
                start = st
        else:
            if start is not None:
                cr.append((start, st))
                start = None

    with tile.TileContext(nc) as tc, ExitStack() as ctx:
        const = ctx.enter_context(tc.tile_pool(name="const", bufs=1))
        persist = ctx.enter_context(tc.tile_pool(name="persist", bufs=1))
        kvpool = ctx.enter_context(tc.tile_pool(name="kvpool", bufs=1))
        wopool = ctx.enter_context(tc.tile_pool(name="wopool", bufs=1))
        agpool = ctx.enter_context(tc.tile_pool(name="agpool", bufs=1))
        dram = ctx.enter_context(tc.tile_pool(name="dram", bufs=1, space="DRAM"))

        ag_in = [dram.tile([2 * HD, L], bf, tag=f"agin{pi}", name=f"ag_in{pi}")
                 for pi in range(2)]
        ag_out = [dram.tile([NCORES * 2 * HD, L], bf, tag=f"agout{pi}",
                            name=f"ag_out{pi}", addr_space="Shared")
                  for pi in range(2)]

        # persistent buffers
        K_T = kvpool.tile([128, S_MAX], bf, tag="kt", name="K_T")   # [d, s]
        v_sb = kvpool.tile([128, S_MAX], bf, tag="v", name="v_sb")  # [s, d] tiles
        qpair = [persist.tile([128, 2, L], bf, tag=f"qp{pi}", name=f"qpair{pi}")
                 for pi in range(2)]

        x_r = x_d.rearrange("(t p) l -> p t l", p=128)
        wk_r = wkT_d.rearrange("(t p) d -> p t d", p=128)
        wv_r = wvT_d.rearrange("(t p) d -> p t d", p=128)
        vc_r = vc_d.rearrange("(t p) d -> p t d", p=128)

        scopeA = ExitStack()
        with scopeA:
            xpool = scopeA.enter_context(tc.tile_pool(name="xpool", bufs=1))
            wqpool = scopeA.enter_context(tc.tile_pool(name="wqpool", bufs=6))
            tmppool = scopeA.enter_context(tc.tile_pool(name="tmppool", bufs=4))
            pp = scopeA.enter_context(tc.tile_pool(name="pp", bufs=1, space="PSUM"))

            # ---- q projections first: PE starts as soon as x0/wq0 land ----
            x_sb = xpool.tile([128, NT, L], bf, tag="x", name="x_sb")
            wk_sb = xpool.tile([128, NT, HD], bf, tag="wk", name="wk_sb")
            wv_sb = xpool.tile([128, NT, HD], bf, tag="wv", name="wv_sb")
            q_ps = [pp.tile([128, L], f32, tag=f"pq{j}", name=f"q_ps{j}")
                    for j in range(SLOTS)]
            k_ps = pp.tile([128, L], f32, tag="pk", name="k_ps")
            v_ps = pp.tile([128, L], f32, tag="pv", name="v_ps")

            for t in range(NT):
                nc.sync.dma_start(out=x_sb[:, t, :], in_=x_r[:, t, :])
                wqt = wqpool.tile([128, SLOTS * HD], bf, tag="wq", name=f"wqt{t}")
                nc.sync.dma_start(out=wqt[:], in_=wqT_d[t * 128:(t + 1) * 128, :])
                if t == 20:
                    # bulk loads queued behind the first few proj tiles
                    nc.sync.dma_start(out=wk_sb[:], in_=wk_r[:])
                    nc.sync.dma_start(out=wv_sb[:], in_=wv_r[:])
                    trig = const.tile([HD, 4, L], f32, tag="trig", name="trig")
                    nc.sync.dma_start(out=trig[:], in_=trig_d[:])
                    bia = const.tile([HD, 6], f32, tag="bia", name="bia")
                    nc.sync.dma_start(out=bia[:], in_=bias_d[:])
                    idrot = const.tile([HD, 2, HD], bf, tag="idrot", name="idrot")
                    nc.sync.dma_start(out=idrot[:], in_=idrot_d[:])
                    ones32 = const.tile([128, 1], f32, tag="ones32", name="ones32")
                    nc.gpsimd.memset(ones32[:], 1.0)
                    onesr = const.tile([1, 128], f32, tag="onesr", name="onesr")
                    nc.gpsimd.memset(onesr[:], 1.0)
                first, last = t == 0, t == NT - 1
                for j in range(SLOTS):
                    nc.tensor.matmul(q_ps[j][:], lhsT=wqt[:, j * 128:(j + 1) * 128],
                                     rhs=x_sb[:, t, :], start=first, stop=last)
            for t in range(NT):
                nc.tensor.matmul(k_ps[:], lhsT=wk_sb[:, t, :], rhs=x_sb[:, t, :],
                                 start=(t == 0), stop=(t == NT - 1))
            for t in range(NT):
                nc.tensor.matmul(v_ps[:], lhsT=wv_sb[:, t, :], rhs=x_sb[:, t, :],
                                 start=(t == 0), stop=(t == NT - 1))

            # ---- K cache -> K_T via DMA-xbar transpose; V cache straight ----
            for (a, b) in cr:
                nc.sync.dma_start_transpose(out=K_T[:, a * 128:b * 128],
                                            in_=kc_d[a * 128:b * 128, :])
                nc.sync.dma_start(out=v_sb[:, a * 128:b * 128],
                                  in_=vc_r[:, a:b, :])

            qcos, qsin = trig[:, 0, :], trig[:, 1, :]
            kcos, ksin = trig[:, 2, :], trig[:, 3, :]
            ident, rotm = idrot[:, 0, :], idrot[:, 1, :]

            # ---- bias + RoPE (rotate_half as a ±1 permutation matmul) ----
            def rope(dst, raw, cos_t, sin_t):
                rot_ps = pp.tile([128, L], f32, tag="tp", bufs=2, name="rot_ps")
                nc.tensor.matmul(rot_ps[:], lhsT=rotm, rhs=raw[:],
                                 start=True, stop=True)
                t1 = tmppool.tile([128, L], f32, tag="rt1", name="rt1")
                nc.vector.tensor_mul(t1[:], raw[:], cos_t)
                t2 = tmppool.tile([128, L], f32, tag="rt2", name="rt2")
                nc.vector.tensor_mul(t2[:], rot_ps[:], sin_t)
                nc.vector.tensor_add(dst, t1[:], t2[:])

            for j in range(SLOTS):
                q_raw = tmppool.tile([128, L], bf, tag="qraw", bufs=2, name=f"q_raw{j}")
                nc.scalar.activation(q_raw[:], q_ps[j][:], AF.Identity,
                                     bias=bia[:, j:j + 1])
                rope(qpair[j // 2][:, j % 2, :], q_raw, qcos, qsin)

            k_raw = tmppool.tile([128, L], bf, tag="kraw", bufs=1, name="k_raw")
            nc.scalar.activation(k_raw[:], k_ps[:], AF.Identity, bias=bia[:, 4:5])
            rope(K_T[:, cp:cp + L], k_raw, kcos, ksin)

            v_raw = tmppool.tile([128, L], bf, tag="vraw", bufs=1, name="v_raw")
            nc.scalar.activation(v_raw[:], v_ps[:], AF.Identity, bias=bia[:, 5:6])
            for lt in range(L // 128):
                tp = pp.tile([128, 128], bf, tag="tp", bufs=2, name=f"tpv{lt}")
                nc.tensor.transpose(tp[:], v_raw[:, lt * 128:(lt + 1) * 128], ident)
                nc.scalar.copy(v_sb[:, (wt0 + lt) * 128:(wt0 + lt + 1) * 128], tp[:])

        # ---- o_proj weights prefetch (queued after phase-A DMAs) ----
        woT_sb = []
        for gi in range(len(REAL_JC)):
            w = wopool.tile([128, OSH], bf, name=f"woT{gi}")
            nc.sync.dma_start(out=w[:], in_=woT_d[gi * 128:(gi + 1) * 128, :])
            woT_sb.append(w)

        attg = {}

        # ---- attention, slot pairs; den folded on DVE ----
        scopeB = ExitStack()
        with scopeB:
            pa = scopeB.enter_context(tc.tile_pool(name="pa", bufs=1, space="PSUM"))
            ppool = scopeB.enter_context(tc.tile_pool(name="ppool", bufs=5))
            accpool = scopeB.enter_context(tc.tile_pool(name="accpool", bufs=1))
            spool = scopeB.enter_context(tc.tile_pool(name="spool", bufs=2))

            def make_tail(pi, out_e, out_o, acc_e, acc_o):
                def tail():
                    for h, (out_ps, acc) in enumerate(((out_e, acc_e),
                                                       (out_o, acc_o))):
                        j = 2 * pi + h
                        den_ps = pa.tile([1, L], f32, tag="sc", bufs=2,
                                         name=f"den{j}")
                        nc.tensor.matmul(den_ps[:], lhsT=ones32[:], rhs=acc[:],
                                         start=True, stop=True)
                        den_sb = spool.tile([1, L], f32, tag="den_sb",
                                            name=f"den_sb{j}")
                        nc.vector.tensor_copy(den_sb[:], den_ps[:])
                        rec = spool.tile([1, L], f32, tag="rec", name=f"rec{j}")
                        scr = spool.tile([1, L], f32, tag="scr", name=f"scr{j}")
                        nc.vector.reciprocal_approx_accurate(rec[:], den_sb[:],
                                                             scr[:])
                        bc_ps = pa.tile([128, L], f32, tag="sc", bufs=2,
                                        name=f"bc_ps{j}")
                        nc.tensor.matmul(bc_ps[:], lhsT=onesr[:], rhs=rec[:],
                                         start=True, stop=True)
                        bc_sb = spool.tile([128, L], f32, tag="bc_sb",
                                           name=f"bc_sb{j}")
                        nc.vector.tensor_copy(bc_sb[:], bc_ps[:])
                        att = spool.tile([128, L], bf, tag=f"att{j}", bufs=1,
                                         name=f"att{j}")
                        nc.vector.tensor_mul(att[:], out_ps[:], bc_sb[:])
                        nc.sync.dma_start(out=ag_in[pi][h * HD:(h + 1) * HD, :],
                                          in_=att[:])
                    nc.gpsimd.collective_compute(
                        "AllGather",
                        mybir.AluOpType.bypass,
                        replica_groups=[list(range(NCORES))],
                        ins=[ag_in[pi].opt()],
                        outs=[ag_out[pi].opt()],
                    )
                    agv = ag_out[pi].rearrange("(c h p) l -> p c h l",
                                               c=NCORES, h=2, p=128)
                    ag_t = agpool.tile([128, NCORES, 2, L], bf, tag=f"attg{pi}",
                                       name=f"attg{pi}")
                    nc.sync.dma_start(out=ag_t[:], in_=agv)
                    attg[pi] = ag_t
                return tail

            pending = []
            for pi in range(2):
                j0, j1 = 2 * pi, 2 * pi + 1
                out_e = pa.tile([128, L], f32, tag="oute", bufs=2,
                                name=f"out_e{pi}")
                out_o = pa.tile([128, L], f32, tag="outo", bufs=2,
                                name=f"out_o{pi}")
                acc_e = accpool.tile([128, L], f32, tag=f"acc{j0}",
                                     name=f"acc{j0}")
                acc_o = accpool.tile([128, L], f32, tag=f"acc{j1}",
                                     name=f"acc{j1}")
                p_prev = None
                for st in range(ST):
                    if st == 6 and pending:
                        pending.pop(0)()
                    sc = pa.tile([128, 2 * L], f32, tag="sc", bufs=2,
                                 name=f"sc{pi}_{st}")
                    kt = K_T[:, st * 128:(st + 1) * 128]
                    nc.tensor.matmul(sc[:, 0:L], lhsT=kt,
                                     rhs=qpair[pi][:, 0, :], start=True, stop=True)
                    nc.tensor.matmul(sc[:, L:2 * L], lhsT=kt,
                                     rhs=qpair[pi][:, 1, :], start=True, stop=True)
                    p = ppool.tile([128, 2 * L], bf, tag="p", name=f"p{pi}_{st}")
                    nc.scalar.activation(p[:], sc[:], AF.Exp, scale=SCALE)
                    vt = v_sb[:, st * 128:(st + 1) * 128]
                    nc.tensor.matmul(out_e[:], lhsT=vt, rhs=p[:, 0:L],
                                     start=(st == 0), stop=(st == ST - 1))
                    nc.tensor.matmul(out_o[:], lhsT=vt, rhs=p[:, L:2 * L],
                                     start=(st == 0), stop=(st == ST - 1))
                    if st % 2 == 0:
                        p_prev = p
                    else:
                        tbe = ppool.tile([128, L], bf, tag="tb", bufs=4,
                                         name=f"tbe{pi}_{st}")
                        nc.vector.tensor_add(tbe[:], p_prev[:, 0:L], p[:, 0:L])
                        tbo = ppool.tile([128, L], bf, tag="tb", bufs=4,
                                         name=f"tbo{pi}_{st}")
                        nc.vector.tensor_add(tbo[:], p_prev[:, L:2 * L],
                                             p[:, L:2 * L])
                        if st == 1:
                            nc.vector.tensor_copy(acc_e[:], tbe[:])
                            nc.vector.tensor_copy(acc_o[:], tbo[:])
                        else:
                            nc.vector.tensor_add(acc_e[:], acc_e[:], tbe[:])
                            nc.vector.tensor_add(acc_o[:], acc_o[:], tbo[:])
                pending.append(make_tail(pi, out_e, out_o, acc_e, acc_o))
            for t_ in pending:
                t_()

        # ---- o_proj over gathered groups (PSUM banks reused) ----
        scopeC = ExitStack()
        with scopeC:
            po = scopeC.enter_context(tc.tile_pool(name="po", bufs=1, space="PSUM"))
            opool = scopeC.enter_context(tc.tile_pool(name="opool", bufs=2))

            o_ps = [po.tile([OSH // 4, L], f32, tag=f"o{ot}", name=f"o_ps{ot}")
                    for ot in range(4)]
            gi = 0
            NREAL = len(REAL_JC)
            for pi in range(2):
                for c in range(NCORES):
                    for hh in range(2):
                        if _head_of(c, 2 * pi + hh) is None:
                            continue
                        for ot in range(4):
                            m0 = ot * (OSH // 4)
                            nc.tensor.matmul(o_ps[ot][:],
                                             lhsT=woT_sb[gi][:, m0:m0 + OSH // 4],
                                             rhs=attg[pi][:, c, hh, :],
                                             start=(gi == 0),
                                             stop=(gi == NREAL - 1))
                        gi += 1

            for ot in range(4):
                m0 = ot * (OSH // 4)
                osb = opool.tile([OSH // 4, L], f32, tag="osb", name=f"osb{ot}")
                nc.scalar.copy(osb[:], o_ps[ot][:])
                nc.sync.dma_start(out=out_d[m0:m0 + OSH // 4, :], in_=osb[:])

    nc.compile()
    return nc


def _get_prog(cp):
    if cp not in _prog_cache:
        _prog_cache[cp] = _build(cp)
    return _prog_cache[cp]


def _shards(hidden_states, cos, sin, cos_t, sin_t, key_cache, value_cache,
            wq, bq, wk, bk, wv, bv, wo):
    import ml_dtypes
    f = np.float32
    b16 = ml_dtypes.bfloat16
    x = np.ascontiguousarray(hidden_states.reshape(D, L)).astype(b16)
    qcos = np.asarray(cos_t, dtype=f).reshape(HD, L)
    qsin = np.asarray(sin_t, dtype=f).reshape(HD, L)
    kcos = np.asarray(cos, dtype=f).reshape(L, HD).T
    ksin = np.asarray(sin, dtype=f).reshape(L, HD).T
    trig = np.ascontiguousarray(np.stack([qcos, qsin, kcos, ksin], axis=1))
    rotm = np.zeros((HD, HD), dtype=f)   # rot(q) = R @ q; pass R.T as lhsT
    half = HD // 2
    rotm[np.arange(half), np.arange(half) + half] = -1.0
    rotm[np.arange(half) + half, np.arange(half)] = 1.0
    idrot = np.ascontiguousarray(
        np.stack([np.eye(HD, dtype=f), rotm.T], axis=1)).astype(b16)

    maps = []
    for c in range(NCORES):
        kvh = c // 2
        wqT = np.zeros((D, SLOTS * HD), dtype=f)
        biases = np.zeros((HD, 6), dtype=f)
        for s in range(SLOTS):
            h = _head_of(c, s)
            if h is None:
                continue
            wqT[:, s * HD:(s + 1) * HD] = wq[h * HD:(h + 1) * HD, :].T
            biases[:, s] = bq[h * HD:(h + 1) * HD]
        biases[:, 4] = bk[kvh * HD:(kvh + 1) * HD]
        biases[:, 5] = bv[kvh * HD:(kvh + 1) * HD]
        woT = np.empty((H * HD, OSH), dtype=f)
        rows = slice(OSH * c, OSH * (c + 1))
        for gi, (jj, cc) in enumerate(REAL_JC):
            h = _head_of(cc, jj)
            woT[gi * HD:(gi + 1) * HD, :] = wo[rows, h * HD:(h + 1) * HD].T
        maps.append({
            "x": x,
            "wqT": wqT.astype(b16),
            "wkT": np.ascontiguousarray(wk[kvh * HD:(kvh + 1) * HD, :].T).astype(b16),
            "wvT": np.ascontiguousarray(wv[kvh * HD:(kvh + 1) * HD, :].T).astype(b16),
            "kcache": np.ascontiguousarray(key_cache[LI, kvh]).astype(b16),
            "vcache": np.ascontiguousarray(value_cache[LI, kvh]).astype(b16),
            "trig": trig,
            "biases": np.ascontiguousarray(biases),
            "idrot": idrot,
            "woT": woT.astype(b16),
        })
    return maps


def kernel(_trace=False, **inputs):
    from concourse.bass_utils import run_bass_kernel_spmd

    cp = int(np.asarray(inputs["cache_position"]))
    assert cp % 128 == 0 and 0 <= cp <= S_MAX - L, f"unsupported cache_position {cp}"

    maps = _shards(
        inputs["hidden_states"], inputs["cos"], inputs["sin"],
        inputs["cos_t"], inputs["sin_t"],
        inputs["key_cache"], inputs["value_cache"],
        inputs["wq"], inputs["bq"], inputs["wk"], inputs["bk"],
        inputs["wv"], inputs["bv"], inputs["wo"],
    )
    nc = _get_prog(cp)
    res = run_bass_kernel_spmd(nc, maps, core_ids=list(range(NCORES)),
                               trace=_trace)
    out = np.concatenate([r["out"] for r in res.results], axis=0)
    out = out.astype(np.float32).reshape(1, D, 1, L)
    if _trace:
        return out, res
    return out


# revision 20
# speedup vs baseline: 1.0051x; 1.0051x over previous
"""Bass/Tile TRN2 kernel for nn_AttentionANEWraperChannelsFirstWithCache.

Tensor-parallel over heads across 8 NeuronCores:
  - 28 q heads padded to 32 slots (4 per core; odd cores carry 1 zero dummy).
  - core c owns kv head c//2 (each kv head replicated on a core pair).
  - per core: q/k/v projections for own slots, RoPE, in-SBUF cache update
    (K cache transposed to [d, s] via DMA-xbar transpose), attention over the
    full 4096-row cache in [s, l] layout with slots processed in pairs
    (scores/exp at free dim 1024), softmax denominator accumulated on DVE
    with a single fp32 ones-matmul per slot, normalization broadcast on
    GPSIMD.
  - per-slot AllGather of head outputs overlapped with later attention;
    column-parallel o_proj (448 output rows per core) at the end. Host
    concatenates the 8 row shards.

Matmul operands are bf16 (fp32 PSUM accumulation); softmax stats and
normalization stay fp32.
"""

import math
import numpy as np

H, KV, HD, LI = 28, 4, 128, 5
S_MAX, D, L = 4096, 3584, 512
NCORES = 8
SLOTS = 4                  # head slots per core (28 real heads padded to 32)
OSH = D // NCORES          # 448 o_proj output rows per core
NT = D // 128              # 28 contraction tiles over hidden dim
ST = S_MAX // 128          # 32 s-tiles over the cache
SCALE = 1.0 / math.sqrt(HD)


def _head_of(core, slot):
    off = 4 * (core % 2) + slot
    if off >= 7:
        return None                      # dummy slot
    return (core // 2) * 7 + off


# o_proj accumulation order: pair-major (matches the per-pair AllGather),
# then core, then pair half. Slot 3 exists only on even cores.
REAL_JC = [(2 * pi + h, c) for pi in range(2) for c in range(NCORES)
           for h in range(2) if _head_of(c, 2 * pi + h) is not None]


_prog_cache = {}


def _build(cp):
    import concourse.bass as bass
    import concourse.mybir as mybir
    import concourse.tile as tile
    from concourse import bacc
    from contextlib import ExitStack

    f32 = mybir.dt.float32
    bf = mybir.dt.bfloat16
    AF = mybir.ActivationFunctionType
    nc = bacc.Bacc("TRN2", target_bir_lowering=False, debug=False,
                   num_devices=NCORES)

    x_d = nc.dram_tensor("x", [D, L], bf, kind="ExternalInput")
    wqT_d = nc.dram_tensor("wqT", [D, SLOTS * HD], bf, kind="ExternalInput")
    wkT_d = nc.dram_tensor("wkT", [D, HD], bf, kind="ExternalInput")
    wvT_d = nc.dram_tensor("wvT", [D, HD], bf, kind="ExternalInput")
    kc_d = nc.dram_tensor("kcache", [S_MAX, HD], bf, kind="ExternalInput")
    vc_d = nc.dram_tensor("vcache", [S_MAX, HD], bf, kind="ExternalInput")
    trig_d = nc.dram_tensor("trig", [HD, 4, L], f32, kind="ExternalInput")
    bias_d = nc.dram_tensor("biases", [HD, 6], f32, kind="ExternalInput")
    idrot_d = nc.dram_tensor("idrot", [HD, 2, HD], bf, kind="ExternalInput")
    woT_d = nc.dram_tensor("woT", [H * HD, OSH], bf, kind="ExternalInput")
    out_d = nc.dram_tensor("out", [OSH, L], f32, kind="ExternalOutput")

    wt0 = cp // 128                      # first window s-tile
    wset = set(range(wt0, wt0 + L // 128))
    # contiguous cache s-tile ranges outside the update window
    cr = []
    start = None
    for st in range(ST + 1):
        if st < ST and st not in wset:
            if start is None:
                start = st
        else:
            if start is not None:
                cr.append((start, st))
                start = None

    with tile.TileContext(nc) as tc, ExitStack() as ctx:
        const = ctx.enter_context(tc.tile_pool(name="const", bufs=1))
        persist = ctx.enter_context(tc.tile_pool(name="persist", bufs=1))
        kvpool = ctx.enter_context(tc.tile_pool(name="kvpool", bufs=1))
        wopool = ctx.enter_context(tc.tile_pool(name="wopool", bufs=1))
        agpool = ctx.enter_context(tc.tile_pool(name="agpool", bufs=1))
        dram = ctx.enter_context(tc.tile_pool(name="dram", bufs=1, space="DRAM"))

        ag_in = [dram.tile([2 * HD, L], bf, tag=f"agin{pi}", name=f"ag_in{pi}")
                 for pi in range(2)]
        ag_out = [dram.tile([NCORES * 2 * HD, L], bf, tag=f"agout{pi}",
                            name=f"ag_out{pi}", addr_space="Shared")
                  for pi in range(2)]

        # persistent buffers
        K_T = kvpool.tile([128, S_MAX], bf, tag="kt", name="K_T")   # [d, s]
        v_sb = kvpool.tile([128, S_MAX], bf, tag="v", name="v_sb")  # [s, d] tiles
        qpair = [persist.tile([128, 2, L], bf, tag=f"qp{pi}", name=f"qpair{pi}")
                 for pi in range(2)]

        x_r = x_d.rearrange("(t p) l -> p t l", p=128)
        wk_r = wkT_d.rearrange("(t p) d -> p t d", p=128)
        wv_r = wvT_d.rearrange("(t p) d -> p t d", p=128)
        vc_r = vc_d.rearrange("(t p) d -> p t d", p=128)

        scopeA = ExitStack()
        with scopeA:
            xpool = scopeA.enter_context(tc.tile_pool(name="xpool", bufs=1))
            wqpool = scopeA.enter_context(tc.tile_pool(name="wqpool", bufs=6))
            tmppool = scopeA.enter_context(tc.tile_pool(name="tmppool", bufs=4))
            pp = scopeA.enter_context(tc.tile_pool(name="pp", bufs=1, space="PSUM"))

            # ---- q projections first: PE starts as soon as x0/wq0 land ----
            x_sb = xpool.tile([128, NT, L], bf, tag="x", name="x_sb")
            wk_sb = xpool.tile([128, NT, HD], bf, tag="wk", name="wk_sb")
            wv_sb = xpool.tile([128, NT, HD], bf, tag="wv", name="wv_sb")
            q_ps = [pp.tile([128, L], f32, tag=f"pq{j}", name=f"q_ps{j}")
                    for j in range(SLOTS)]
            k_ps = pp.tile([128, L], f32, tag="pk", name="k_ps")
            v_ps = pp.tile([128, L], f32, tag="pv", name="v_ps")

            for t in range(NT):
                nc.sync.dma_start(out=x_sb[:, t, :], in_=x_r[:, t, :])
                wqt = wqpool.tile([128, SLOTS * HD], bf, tag="wq", name=f"wqt{t}")
                nc.sync.dma_start(out=wqt[:], in_=wqT_d[t * 128:(t + 1) * 128, :])
                if t == 20:
                    # bulk loads queued behind the first few proj tiles
                    nc.sync.dma_start(out=wk_sb[:], in_=wk_r[:])
                    nc.sync.dma_start(out=wv_sb[:], in_=wv_r[:])
                    trig = const.tile([HD, 4, L], f32, tag="trig", name="trig")
                    nc.sync.dma_start(out=trig[:], in_=trig_d[:])
                    bia = const.tile([HD, 6], f32, tag="bia", name="bia")
                    nc.sync.dma_start(out=bia[:], in_=bias_d[:])
                    idrot = const.tile([HD, 2, HD], bf, tag="idrot", name="idrot")
                    nc.sync.dma_start(out=idrot[:], in_=idrot_d[:])
                    ones32 = const.tile([128, 1], f32, tag="ones32", name="ones32")
                    nc.gpsimd.memset(ones32[:], 1.0)
                    onesr = const.tile([1, 128], f32, tag="onesr", name="onesr")
                    nc.gpsimd.memset(onesr[:], 1.0)
                first, last = t == 0, t == NT - 1
                for j in range(SLOTS):
                    nc.tensor.matmul(q_ps[j][:], lhsT=wqt[:, j * 128:(j + 1) * 128],
                                     rhs=x_sb[:, t, :], start=first, stop=last)
            for t in range(NT):
                nc.tensor.matmul(k_ps[:], lhsT=wk_sb[:, t, :], rhs=x_sb[:, t, :],
                                 start=(t == 0), stop=(t == NT - 1))
            for t in range(NT):
                nc.tensor.matmul(v_ps[:], lhsT=wv_sb[:, t, :], rhs=x_sb[:, t, :],
                                 start=(t == 0), stop=(t == NT - 1))

            # ---- K cache -> K_T via DMA-xbar transpose; V cache straight ----
            for (a, b) in cr:
                nc.sync.dma_start_transpose(out=K_T[:, a * 128:b * 128],
                                            in_=kc_d[a * 128:b * 128, :])
                nc.sync.dma_start(out=v_sb[:, a * 128:b * 128],
                                  in_=vc_r[:, a:b, :])

            qcos, qsin = trig[:, 0, :], trig[:, 1, :]
            kcos, ksin = trig[:, 2, :], trig[:, 3, :]
            ident, rotm = idrot[:, 0, :], idrot[:, 1, :]

            # ---- bias + RoPE (rotate_half as a ±1 permutation matmul) ----
            def rope(dst, raw, cos_t, sin_t):
                rot_ps = pp.tile([128, L], f32, tag="tp", bufs=2, name="rot_ps")
                nc.tensor.matmul(rot_ps[:], lhsT=rotm, rhs=raw[:],
                                 start=True, stop=True)
                t1 = tmppool.tile([128, L], f32, tag="rt1", name="rt1")
                nc.vector.tensor_mul(t1[:], raw[:], cos_t)
                t2 = tmppool.tile([128, L], f32, tag="rt2", name="rt2")
                nc.vector.tensor_mul(t2[:], rot_ps[:], sin_t)
                nc.vector.tensor_add(dst, t1[:], t2[:])

            for j in range(SLOTS):
                q_raw = tmppool.tile([128, L], bf, tag="qraw", bufs=2, name=f"q_raw{j}")
                nc.scalar.activation(q_raw[:], q_ps[j][:], AF.Identity,
                                     bias=bia[:, j:j + 1])
                rope(qpair[j // 2][:, j % 2, :], q_raw, qcos, qsin)

            k_raw = tmppool.tile([128, L], bf, tag="kraw", bufs=1, name="k_raw")
            nc.scalar.activation(k_raw[:], k_ps[:], AF.Identity, bias=bia[:, 4:5])
            rope(K_T[:, cp:cp + L], k_raw, kcos, ksin)

            v_raw = tmppool.tile([128, L], bf, tag="vraw", bufs=1, name="v_raw")
            nc.scalar.activation(v_raw[:], v_ps[:], AF.Identity, bias=bia[:, 5:6])
            for lt in range(L // 128):
                tp = pp.tile([128, 128], bf, tag="tp", bufs=2, name=f"tpv{lt}")
                nc.tensor.transpose(tp[:], v_raw[:, lt * 128:(lt + 1) * 128], ident)
                nc.scalar.copy(v_sb[:, (wt0 + lt) * 128:(wt0 + lt + 1) * 128], tp[:])

        # ---- o_proj weights prefetch (queued after phase-A DMAs) ----
        woT_sb = []
        for gi in range(len(REAL_JC)):
            w = wopool.tile([128, OSH], bf, name=f"woT{gi}")
            nc.sync.dma_start(out=w[:], in_=woT_d[gi * 128:(gi + 1) * 128, :])
            woT_sb.append(w)

        attg = {}

        # ---- attention, slot pairs; den folded on DVE ----
        scopeB = ExitStack()
        with scopeB:
            pa = scopeB.enter_context(tc.tile_pool(name="pa", bufs=1, space="PSUM"))
            ppool = scopeB.enter_context(tc.tile_pool(name="ppool", bufs=5))
            accpool = scopeB.enter_context(tc.tile_pool(name="accpool", bufs=1))
            spool = scopeB.enter_context(tc.tile_pool(name="spool", bufs=2))

            def make_tail(pi, out_e, out_o, acc_e, acc_o):
                def tail():
                    for h, (out_ps, acc) in enumerate(((out_e, acc_e),
                                                       (out_o, acc_o))):
                        j = 2 * pi + h
                        den_ps = pa.tile([1, L], f32, tag="sc", bufs=2,
                                         name=f"den{j}")
                        nc.tensor.matmul(den_ps[:], lhsT=ones32[:], rhs=acc[:],
                                         start=True, stop=True)
                        den_sb = spool.tile([1, L], f32, tag="den_sb",
                                            name=f"den_sb{j}")
                        nc.vector.tensor_copy(den_sb[:], den_ps[:])
                        rec = spool.tile([1, L], f32, tag="rec", name=f"rec{j}")
                        scr = spool.tile([1, L], f32, tag="scr", name=f"scr{j}")
                        nc.vector.reciprocal_approx_accurate(rec[:], den_sb[:],
                                                             scr[:])
                        bc_ps = pa.tile([128, L], f32, tag="sc", bufs=2,
                                        name=f"bc_ps{j}")
                        nc.tensor.matmul(bc_ps[:], lhsT=onesr[:], rhs=rec[:],
                                         start=True, stop=True)
                        bc_sb = spool.tile([128, L], f32, tag="bc_sb",
                                           name=f"bc_sb{j}")
                        nc.vector.tensor_copy(bc_sb[:], bc_ps[:])
                        att = spool.tile([128, L], bf, tag=f"att{j}", bufs=1,
                                         name=f"att{j}")
                        nc.vector.tensor_mul(att[:], out_ps[:], bc_sb[:])
                        nc.sync.dma_start(out=ag_in[pi][h * HD:(h + 1) * HD, :],
                                          in_=att[:])
                    nc.gpsimd.collective_compute(
                        "AllGather",
                        mybir.AluOpType.bypass,
                        replica_groups=[list(range(NCORES))],
                        ins=[ag_in[pi].opt()],
                        outs=[ag_out[pi].opt()],
                    )
                    agv = ag_out[pi].rearrange("(c h p) l -> p c h l",
                                               c=NCORES, h=2, p=128)
                    ag_t = agpool.tile([128, NCORES, 2, L], bf, tag=f"attg{pi}",
                                       name=f"attg{pi}")
                    hc = NCORES // 2
                    nc.sync.dma_start(out=ag_t[:, 0:hc], in_=agv[:, 0:hc])
                    nc.sync.dma_start(out=ag_t[:, hc:], in_=agv[:, hc:])
                    attg[pi] = ag_t
                return tail

            pending = []
            for pi in range(2):
                j0, j1 = 2 * pi, 2 * pi + 1
                out_e = pa.tile([128, L], f32, tag="oute", bufs=2,
                                name=f"out_e{pi}")
                out_o = pa.tile([128, L], f32, tag="outo", bufs=2,
                                name=f"out_o{pi}")
                acc_e = accpool.tile([128, L], f32, tag=f"acc{j0}",
                                     name=f"acc{j0}")
                acc_o = accpool.tile([128, L], f32, tag=f"acc{j1}",
                                     name=f"acc{j1}")
                p_prev = None
                for st in range(ST):
                    if st == 2 and pending:
                        pending.pop(0)()
                    sc = pa.tile([128, 2 * L], f32, tag="sc", bufs=2,
                                 name=f"sc{pi}_{st}")
                    kt = K_T[:, st * 128:(st + 1) * 128]
                    nc.tensor.matmul(sc[:, 0:L], lhsT=kt,
                                     rhs=qpair[pi][:, 0, :], start=True, stop=True)
                    nc.tensor.matmul(sc[:, L:2 * L], lhsT=kt,
                                     rhs=qpair[pi][:, 1, :], start=True, stop=True)
                    p = ppool.tile([128, 2 * L], bf, tag="p", name=f"p{pi}_{st}")
                    nc.scalar.activation(p[:], sc[:], AF.Exp, scale=SCALE)
                    vt = v_sb[:, st * 128:(st + 1) * 128]
                    nc.tensor.matmul(out_e[:], lhsT=vt, rhs=p[:, 0:L],
                                     start=(st == 0), stop=(st == ST - 1))
                    nc.tensor.matmul(out_o[:], lhsT=vt, rhs=p[:, L:2 * L],
                                     start=(st == 0), stop=(st == ST - 1))
                    if st % 2 == 0:
                        p_prev = p
                    else:
                        tbe = ppool.tile([128, L], bf, tag="tb", bufs=4,
                                         name=f"tbe{pi}_{st}")
                        nc.vector.tensor_add(tbe[:], p_prev[:, 0:L], p[:, 0:L])
                        tbo = ppool.tile([128, L], bf, tag="tb", bufs=4,
                                         name=f"tbo{pi}_{st}")
                        nc.vector.tensor_add(tbo[:], p_prev[:, L:2 * L],
                                             p[:, L:2 * L])
                        if st == 1:
                            nc.vector.tensor_copy(acc_e[:], tbe[:])
                            nc.vector.tensor_copy(acc_o[:], tbo[:])
                        else:
                            nc.vector.tensor_add(acc_e[:], acc_e[:], tbe[:])
                            nc.vector.tensor_add(acc_o[:], acc_o[:], tbo[:])
                pending.append(make_tail(pi, out_e, out_o, acc_e, acc_o))
            for t_ in pending:
                t_()

        # ---- o_proj over gathered groups (PSUM banks reused) ----
        scopeC = ExitStack()
        with scopeC:
            po = scopeC.enter_context(tc.tile_pool(name="po", bufs=1, space="PSUM"))
            opool = scopeC.enter_context(tc.tile_pool(name="opool", bufs=2))

            o_ps = [po.tile([OSH // 4, L], f32, tag=f"o{ot}", name=f"o_ps{ot}")
                    for ot in range(4)]
            gi = 0
            NREAL = len(REAL_JC)
            for pi in range(2):
                for c in range(NCORES):
                    for hh in range(2):
                        if _head_of(c, 2 * pi + hh) is None:
                            continue
                        for ot in range(4):
                            m0 = ot * (OSH // 4)
                            nc.tensor.matmul(o_ps[ot][:],
                                             lhsT=woT_sb[gi][:, m0:m0 + OSH // 4],
                                             rhs=attg[pi][:, c, hh, :],
                                             start=(gi == 0),
                                             stop=(gi == NREAL - 1))
                        gi += 1

            for ot in range(4):
                m0 = ot * (OSH // 4)
                osb = opool.tile([OSH // 4, L], f32, tag="osb", name=f"osb{ot}")
                nc.scalar.copy(osb[:], o_ps[ot][:])
                nc.sync.dma_start(out=out_d[m0:m0 + OSH // 4, :], in_=osb[:])

    nc.compile()
    return nc


def _get_prog(cp):
    if cp not in _prog_cache:
        _prog_cache[cp] = _build(cp)
    return _prog_cache[cp]


def _shards(hidden_states, cos, sin, cos_t, sin_t, key_cache, value_cache,
            wq, bq, wk, bk, wv, bv, wo):
    import ml_dtypes
    f = np.float32
    b16 = ml_dtypes.bfloat16
    x = np.ascontiguousarray(hidden_states.reshape(D, L)).astype(b16)
    qcos = np.asarray(cos_t, dtype=f).reshape(HD, L)
    qsin = np.asarray(sin_t, dtype=f).reshape(HD, L)
    kcos = np.asarray(cos, dtype=f).reshape(L, HD).T
    ksin = np.asarray(sin, dtype=f).reshape(L, HD).T
    trig = np.ascontiguousarray(np.stack([qcos, qsin, kcos, ksin], axis=1))
    rotm = np.zeros((HD, HD), dtype=f)   # rot(q) = R @ q; pass R.T as lhsT
    half = HD // 2
    rotm[np.arange(half), np.arange(half) + half] = -1.0
    rotm[np.arange(half) + half, np.arange(half)] = 1.0
    idrot = np.ascontiguousarray(
        np.stack([np.eye(HD, dtype=f), rotm.T], axis=1)).astype(b16)

    maps = []
    for c in range(NCORES):
        kvh = c // 2
        wqT = np.zeros((D, SLOTS * HD), dtype=f)
        biases = np.zeros((HD, 6), dtype=f)
        for s in range(SLOTS):
            h = _head_of(c, s)
            if h is None:
                continue
            wqT[:, s * HD:(s + 1) * HD] = wq[h * HD:(h + 1) * HD, :].T
            biases[:, s] = bq[h * HD:(h + 1) * HD]
        biases[:, 4] = bk[kvh * HD:(kvh + 1) * HD]
        biases[:, 5] = bv[kvh * HD:(kvh + 1) * HD]
        woT = np.empty((H * HD, OSH), dtype=f)
        rows = slice(OSH * c, OSH * (c + 1))
        for gi, (jj, cc) in enumerate(REAL_JC):
            h = _head_of(cc, jj)
            woT[gi * HD:(gi + 1) * HD, :] = wo[rows, h * HD:(h + 1) * HD].T
        maps.append({
            "x": x,
            "wqT": wqT.astype(b16),
            "wkT": np.ascontiguousarray(wk[kvh * HD:(kvh + 1) * HD, :].T).astype(b16),
            "wvT": np.ascontiguousarray(wv[kvh * HD:(kvh + 1) * HD, :].T).astype(b16),
            "kcache": np.ascontiguousarray(key_cache[LI, kvh]).astype(b16),
            "vcache": np.ascontiguousarray(value_cache[LI, kvh]).astype(b16),
            "trig": trig,
            "biases": np.ascontiguousarray(biases),
            "idrot": idrot,
            "woT": woT.astype(b16),
        })
    return maps


def kernel(_trace=False, **inputs):
    from concourse.bass_utils import run_bass_kernel_spmd

    cp = int(np.asarray(inputs["cache_position"]))
    assert cp % 128 == 0 and 0 <= cp <= S_MAX - L, f"unsupported cache_position {cp}"

    maps = _shards(
        inputs["hidden_states"], inputs["cos"], inputs["sin"],
        inputs["cos_t"], inputs["sin_t"],
        inputs["key_cache"], inputs["value_cache"],
        inputs["wq"], inputs["bq"], inputs["wk"], inputs["bk"],
        inputs["wv"], inputs["bv"], inputs["wo"],
    )
    nc = _get_prog(cp)
    res = run_bass_kernel_spmd(nc, maps, core_ids=list(range(NCORES)),
                               trace=_trace)
    out = np.concatenate([r["out"] for r in res.results], axis=0)
    out = out.astype(np.float32).reshape(1, D, 1, L)
    if _trace:
        return out, res
    return out


# revision 22
# speedup vs baseline: 1.0950x; 1.0894x over previous
"""Bass/Tile TRN2 kernel for nn_AttentionANEWraperChannelsFirstWithCache.

Tensor-parallel over heads across 8 NeuronCores:
  - 28 q heads padded to 32 slots (4 per core; odd cores carry 1 zero dummy).
  - core c owns kv head c//2 (each kv head replicated on a core pair).
  - per core: q/k/v projections for own slots, RoPE, in-SBUF cache update
    (K cache transposed to [d, s] via DMA-xbar transpose), attention over the
    full 4096-row cache in [s, l] layout with slots processed in pairs
    (scores/exp at free dim 1024), softmax denominator accumulated on DVE
    with a single fp32 ones-matmul per slot, normalization broadcast on
    GPSIMD.
  - per-slot AllGather of head outputs overlapped with later attention;
    column-parallel o_proj (448 output rows per core) at the end. Host
    concatenates the 8 row shards.

Matmul operands are bf16 (fp32 PSUM accumulation); softmax stats and
normalization stay fp32.
"""

import math
import numpy as np

H, KV, HD, LI = 28, 4, 128, 5
S_MAX, D, L = 4096, 3584, 512
NCORES = 8
SLOTS = 4                  # head slots per core (28 real heads padded to 32)
OSH = D // NCORES          # 448 o_proj output rows per core
NT = D // 128              # 28 contraction tiles over hidden dim
ST = S_MAX // 128          # 32 s-tiles over the cache
SCALE = 1.0 / math.sqrt(HD)


def _head_of(core, slot):
    off = 4 * (core % 2) + slot
    if off >= 7:
        return None                      # dummy slot
    return (core // 2) * 7 + off


# o_proj accumulation order: pair-major (matches the per-pair AllGather),
# then core, then pair half. Slot 3 exists only on even cores.
GROUPS = [(0, 1), (2, 3)]
REAL_JC = [(j, c) for g in GROUPS for c in range(NCORES)
           for j in g if _head_of(c, j) is not None]


_prog_cache = {}


def _build(cp):
    import concourse.bass as bass
    import concourse.mybir as mybir
    import concourse.tile as tile
    from concourse import bacc
    from contextlib import ExitStack

    f32 = mybir.dt.float32
    bf = mybir.dt.bfloat16
    AF = mybir.ActivationFunctionType
    nc = bacc.Bacc("TRN2", target_bir_lowering=False, debug=False,
                   num_devices=NCORES)

    x_d = nc.dram_tensor("x", [D, L], bf, kind="ExternalInput")
    wqT_d = nc.dram_tensor("wqT", [D, SLOTS * HD], bf, kind="ExternalInput")
    wkT_d = nc.dram_tensor("wkT", [D, HD], bf, kind="ExternalInput")
    wvT_d = nc.dram_tensor("wvT", [D, HD], bf, kind="ExternalInput")
    kc_d = nc.dram_tensor("kcache", [S_MAX, HD], bf, kind="ExternalInput")
    vc_d = nc.dram_tensor("vcache", [S_MAX, HD], bf, kind="ExternalInput")
    trig_d = nc.dram_tensor("trig", [HD, 4, L], f32, kind="ExternalInput")
    bias_d = nc.dram_tensor("biases", [HD, 6], f32, kind="ExternalInput")
    idrot_d = nc.dram_tensor("idrot", [HD, 2, HD], bf, kind="ExternalInput")
    woT_d = nc.dram_tensor("woT", [H * HD, OSH], bf, kind="ExternalInput")
    out_d = nc.dram_tensor("out", [OSH, L], f32, kind="ExternalOutput")

    wt0 = cp // 128                      # first window s-tile
    wset = set(range(wt0, wt0 + L // 128))
    # contiguous cache s-tile ranges outside the update window
    cr = []
    start = None
    for st in range(ST + 1):
        if st < ST and st not in wset:
            if start is None:
                start = st
        else:
            if start is not None:
                cr.append((start, st))
                start = None

    with tile.TileContext(nc) as tc, ExitStack() as ctx:
        const = ctx.enter_context(tc.tile_pool(name="const", bufs=1))
        persist = ctx.enter_context(tc.tile_pool(name="persist", bufs=1))
        kvpool = ctx.enter_context(tc.tile_pool(name="kvpool", bufs=1))
        wopool = ctx.enter_context(tc.tile_pool(name="wopool", bufs=1))
        agpool = ctx.enter_context(tc.tile_pool(name="agpool", bufs=1))
        dram = ctx.enter_context(tc.tile_pool(name="dram", bufs=1, space="DRAM"))

        ag_in = [dram.tile([len(g) * HD, L], bf, tag=f"agin{gi}",
                           name=f"ag_in{gi}")
                 for gi, g in enumerate(GROUPS)]
        ag_out = [dram.tile([NCORES * len(g) * HD, L], bf, tag=f"agout{gi}",
                            name=f"ag_out{gi}", addr_space="Shared")
                  for gi, g in enumerate(GROUPS)]

        # persistent buffers
        K_T = kvpool.tile([128, S_MAX], bf, tag="kt", name="K_T")   # [d, s]
        v_sb = kvpool.tile([128, S_MAX], bf, tag="v", name="v_sb")  # [s, d] tiles
        qpair = persist.tile([128, 2, L], bf, tag="qp", name="qpair")
        q2 = persist.tile([128, L], bf, tag="q2", name="q2_sb")
        q3 = persist.tile([128, L], bf, tag="q3", name="q3_sb")
        q_dst = [qpair[:, 0, :], qpair[:, 1, :], q2[:], q3[:]]

        x_r = x_d.rearrange("(t p) l -> p t l", p=128)
        wk_r = wkT_d.rearrange("(t p) d -> p t d", p=128)
        wv_r = wvT_d.rearrange("(t p) d -> p t d", p=128)
        vc_r = vc_d.rearrange("(t p) d -> p t d", p=128)

        scopeA = ExitStack()
        with scopeA:
            xpool = scopeA.enter_context(tc.tile_pool(name="xpool", bufs=1))
            wqpool = scopeA.enter_context(tc.tile_pool(name="wqpool", bufs=6))
            tmppool = scopeA.enter_context(tc.tile_pool(name="tmppool", bufs=4))
            pp = scopeA.enter_context(tc.tile_pool(name="pp", bufs=1, space="PSUM"))

            # ---- q projections first: PE starts as soon as x0/wq0 land ----
            x_sb = xpool.tile([128, NT, L], bf, tag="x", name="x_sb")
            wk_sb = xpool.tile([128, NT, HD], bf, tag="wk", name="wk_sb")
            wv_sb = xpool.tile([128, NT, HD], bf, tag="wv", name="wv_sb")
            q_ps = [pp.tile([128, L], f32, tag=f"pq{j}", name=f"q_ps{j}")
                    for j in range(SLOTS)]
            k_ps = pp.tile([128, L], f32, tag="pk", name="k_ps")
            v_ps = pp.tile([128, L], f32, tag="pv", name="v_ps")

            for t in range(NT):
                nc.sync.dma_start(out=x_sb[:, t, :], in_=x_r[:, t, :])
                wqt = wqpool.tile([128, SLOTS * HD], bf, tag="wq", name=f"wqt{t}")
                nc.sync.dma_start(out=wqt[:], in_=wqT_d[t * 128:(t + 1) * 128, :])
                if t == 20:
                    # bulk loads queued behind the first few proj tiles
                    nc.sync.dma_start(out=wk_sb[:], in_=wk_r[:])
                    nc.sync.dma_start(out=wv_sb[:], in_=wv_r[:])
                    trig = const.tile([HD, 4, L], f32, tag="trig", name="trig")
                    nc.sync.dma_start(out=trig[:], in_=trig_d[:])
                    bia = const.tile([HD, 6], f32, tag="bia", name="bia")
                    nc.sync.dma_start(out=bia[:], in_=bias_d[:])
                    idrot = const.tile([HD, 2, HD], bf, tag="idrot", name="idrot")
                    nc.sync.dma_start(out=idrot[:], in_=idrot_d[:])
                    ones_bf = const.tile([128, 1], bf, tag="ones_bf", name="ones_bf")
                    nc.gpsimd.memset(ones_bf[:], 1.0)
                    onesr_bf = const.tile([1, 128], bf, tag="onesr_bf", name="onesr_bf")
                    nc.gpsimd.memset(onesr_bf[:], 1.0)
                first, last = t == 0, t == NT - 1
                for j in range(SLOTS):
                    nc.tensor.matmul(q_ps[j][:], lhsT=wqt[:, j * 128:(j + 1) * 128],
                                     rhs=x_sb[:, t, :], start=first, stop=last)
            for t in range(NT):
                nc.tensor.matmul(k_ps[:], lhsT=wk_sb[:, t, :], rhs=x_sb[:, t, :],
                                 start=(t == 0), stop=(t == NT - 1))
            for t in range(NT):
                nc.tensor.matmul(v_ps[:], lhsT=wv_sb[:, t, :], rhs=x_sb[:, t, :],
                                 start=(t == 0), stop=(t == NT - 1))

            # ---- K cache -> K_T via DMA-xbar transpose; V cache straight ----
            for (a, b) in cr:
                nc.sync.dma_start_transpose(out=K_T[:, a * 128:b * 128],
                                            in_=kc_d[a * 128:b * 128, :])
                nc.sync.dma_start(out=v_sb[:, a * 128:b * 128],
                                  in_=vc_r[:, a:b, :])

            qcos, qsin = trig[:, 0, :], trig[:, 1, :]
            kcos, ksin = trig[:, 2, :], trig[:, 3, :]
            ident, rotm = idrot[:, 0, :], idrot[:, 1, :]

            # ---- bias + RoPE (rotate_half as a ±1 permutation matmul) ----
            def rope(dst, raw, cos_t, sin_t):
                rot_ps = pp.tile([128, L], f32, tag="tp", bufs=2, name="rot_ps")
                nc.tensor.matmul(rot_ps[:], lhsT=rotm, rhs=raw[:],
                                 start=True, stop=True)
                t1 = tmppool.tile([128, L], f32, tag="rt1", name="rt1")
                nc.vector.tensor_mul(t1[:], raw[:], cos_t)
                t2 = tmppool.tile([128, L], f32, tag="rt2", name="rt2")
                nc.vector.tensor_mul(t2[:], rot_ps[:], sin_t)
                nc.vector.tensor_add(dst, t1[:], t2[:])

            for j in range(SLOTS):
                q_raw = tmppool.tile([128, L], bf, tag="qraw", bufs=2, name=f"q_raw{j}")
                nc.scalar.activation(q_raw[:], q_ps[j][:], AF.Identity,
                                     bias=bia[:, j:j + 1])
                rope(q_dst[j], q_raw, qcos, qsin)

            k_raw = tmppool.tile([128, L], bf, tag="kraw", bufs=1, name="k_raw")
            nc.scalar.activation(k_raw[:], k_ps[:], AF.Identity, bias=bia[:, 4:5])
            rope(K_T[:, cp:cp + L], k_raw, kcos, ksin)

            v_raw = tmppool.tile([128, L], bf, tag="vraw", bufs=1, name="v_raw")
            nc.scalar.activation(v_raw[:], v_ps[:], AF.Identity, bias=bia[:, 5:6])
            for lt in range(L // 128):
                tp = pp.tile([128, 128], bf, tag="tp", bufs=2, name=f"tpv{lt}")
                nc.tensor.transpose(tp[:], v_raw[:, lt * 128:(lt + 1) * 128], ident)
                nc.scalar.copy(v_sb[:, (wt0 + lt) * 128:(wt0 + lt + 1) * 128], tp[:])

        # ---- o_proj weights prefetch (queued after phase-A DMAs) ----
        woT_sb = []
        for gi in range(len(REAL_JC)):
            w = wopool.tile([128, OSH], bf, name=f"woT{gi}")
            nc.sync.dma_start(out=w[:], in_=woT_d[gi * 128:(gi + 1) * 128, :])
            woT_sb.append(w)

        attg = {}

        # ---- attention by gather groups; den folded on DVE (all-bf16 MMs) ----
        attg = {}
        scopeB = ExitStack()
        with scopeB:
            pa = scopeB.enter_context(tc.tile_pool(name="pa", bufs=1, space="PSUM"))
            ppool = scopeB.enter_context(tc.tile_pool(name="ppool", bufs=5))
            accpool = scopeB.enter_context(tc.tile_pool(name="accpool", bufs=1))
            spool = scopeB.enter_context(tc.tile_pool(name="spool", bufs=2))

            def make_tail(gi, slots_g, outs, accs):
                def tail():
                    for h, j in enumerate(slots_g):
                        acc_bf = ppool.tile([128, L], bf, tag="accbf", bufs=2,
                                            name=f"acc_bf{j}")
                        nc.vector.tensor_copy(acc_bf[:], accs[h][:])
                        den_ps = pa.tile([1, L], f32, tag="sc", bufs=2,
                                         name=f"den{j}")
                        nc.tensor.matmul(den_ps[:], lhsT=ones_bf[:],
                                         rhs=acc_bf[:], start=True, stop=True)
                        den_sb = spool.tile([1, L], f32, tag="den_sb",
                                            name=f"den_sb{j}")
                        nc.vector.tensor_copy(den_sb[:], den_ps[:])
                        rec = spool.tile([1, L], f32, tag="rec", name=f"rec{j}")
                        scr = spool.tile([1, L], f32, tag="scr", name=f"scr{j}")
                        nc.vector.reciprocal_approx_accurate(rec[:], den_sb[:],
                                                             scr[:])
                        rec_bf = spool.tile([1, L], bf, tag="rec_bf",
                                            name=f"rec_bf{j}")
                        nc.vector.tensor_copy(rec_bf[:], rec[:])
                        bc_ps = pa.tile([128, L], f32, tag="sc", bufs=2,
                                        name=f"bc_ps{j}")
                        nc.tensor.matmul(bc_ps[:], lhsT=onesr_bf[:],
                                         rhs=rec_bf[:], start=True, stop=True)
                        bc_sb = spool.tile([128, L], f32, tag="bc_sb",
                                           name=f"bc_sb{j}")
                        nc.vector.tensor_copy(bc_sb[:], bc_ps[:])
                        att = spool.tile([128, L], bf, tag=f"att{j}", bufs=1,
                                         name=f"att{j}")
                        nc.vector.tensor_mul(att[:], outs[h][:], bc_sb[:])
                        nc.sync.dma_start(out=ag_in[gi][h * HD:(h + 1) * HD, :],
                                          in_=att[:])
                    nc.gpsimd.collective_compute(
                        "AllGather",
                        mybir.AluOpType.bypass,
                        replica_groups=[list(range(NCORES))],
                        ins=[ag_in[gi].opt()],
                        outs=[ag_out[gi].opt()],
                    )
                    nh = len(slots_g)
                    agv = ag_out[gi].rearrange("(c h p) l -> p c h l",
                                               c=NCORES, h=nh, p=128)
                    ag_t = agpool.tile([128, NCORES, nh, L], bf,
                                       tag=f"attg{gi}", name=f"attg{gi}")
                    hc = NCORES // 2
                    nc.sync.dma_start(out=ag_t[:, 0:hc], in_=agv[:, 0:hc])
                    nc.sync.dma_start(out=ag_t[:, hc:], in_=agv[:, hc:])
                    attg[gi] = ag_t
                return tail

            q_src = {0: qpair[:, 0, :], 1: qpair[:, 1, :], 2: q2[:], 3: q3[:]}
            pending = []
            for gi, slots_g in enumerate(GROUPS):
                nh = len(slots_g)
                outs = [pa.tile([128, L], f32, tag=f"out{h}", bufs=2,
                                name=f"out{gi}_{h}") for h in range(nh)]
                accs = [accpool.tile([128, L], f32, tag=f"acc{j}",
                                     name=f"acc{j}") for j in slots_g]
                p_prev = None
                for st in range(ST):
                    if st == 2 and pending:
                        pending.pop(0)()
                    sc = pa.tile([128, nh * L], f32, tag="sc", bufs=2,
                                 name=f"sc{gi}_{st}")
                    kt = K_T[:, st * 128:(st + 1) * 128]
                    for h in range(nh):
                        nc.tensor.matmul(sc[:, h * L:(h + 1) * L], lhsT=kt,
                                         rhs=q_src[slots_g[h]],
                                         start=True, stop=True)
                    p = ppool.tile([128, nh * L], bf, tag="p", name=f"p{gi}_{st}")
                    nc.scalar.activation(p[:], sc[:], AF.Exp, scale=SCALE)
                    vt = v_sb[:, st * 128:(st + 1) * 128]
                    for h in range(nh):
                        nc.tensor.matmul(outs[h][:], lhsT=vt,
                                         rhs=p[:, h * L:(h + 1) * L],
                                         start=(st == 0), stop=(st == ST - 1))
                    if st % 2 == 0:
                        p_prev = p
                    else:
                        for h in range(nh):
                            tb = ppool.tile([128, L], bf, tag="tb", bufs=4,
                                            name=f"tb{gi}_{st}_{h}")
                            nc.vector.tensor_add(tb[:],
                                                 p_prev[:, h * L:(h + 1) * L],
                                                 p[:, h * L:(h + 1) * L])
                            if st == 1:
                                nc.vector.tensor_copy(accs[h][:], tb[:])
                            else:
                                nc.vector.tensor_add(accs[h][:], accs[h][:],
                                                     tb[:])
                pending.append(make_tail(gi, slots_g, outs, accs))
            for t_ in pending:
                t_()

        # ---- o_proj over gathered groups (PSUM banks reused) ----
        scopeC = ExitStack()
        with scopeC:
            po = scopeC.enter_context(tc.tile_pool(name="po", bufs=1, space="PSUM"))
            opool = scopeC.enter_context(tc.tile_pool(name="opool", bufs=2))

            o_ps = [po.tile([OSH // 4, L], f32, tag=f"o{ot}", name=f"o_ps{ot}")
                    for ot in range(4)]
            gi_ = 0
            NREAL = len(REAL_JC)
            for gidx, slots_g in enumerate(GROUPS):
                for c in range(NCORES):
                    for h, j in enumerate(slots_g):
                        if _head_of(c, j) is None:
                            continue
                        for ot in range(4):
                            m0 = ot * (OSH // 4)
                            nc.tensor.matmul(
                                o_ps[ot][:],
                                lhsT=woT_sb[gi_][:, m0:m0 + OSH // 4],
                                rhs=attg[gidx][:, c, h, :],
                                start=(gi_ == 0), stop=(gi_ == NREAL - 1))
                        gi_ += 1

            for ot in range(4):
                m0 = ot * (OSH // 4)
                osb = opool.tile([OSH // 4, L], f32, tag="osb", name=f"osb{ot}")
                nc.scalar.copy(osb[:], o_ps[ot][:])
                nc.sync.dma_start(out=out_d[m0:m0 + OSH // 4, :], in_=osb[:])

    nc.compile()
    return nc


def _get_prog(cp):
    if cp not in _prog_cache:
        _prog_cache[cp] = _build(cp)
    return _prog_cache[cp]


def _shards(hidden_states, cos, sin, cos_t, sin_t, key_cache, value_cache,
            wq, bq, wk, bk, wv, bv, wo):
    import ml_dtypes
    f = np.float32
    b16 = ml_dtypes.bfloat16
    x = np.ascontiguousarray(hidden_states.reshape(D, L)).astype(b16)
    qcos = np.asarray(cos_t, dtype=f).reshape(HD, L)
    qsin = np.asarray(sin_t, dtype=f).reshape(HD, L)
    kcos = np.asarray(cos, dtype=f).reshape(L, HD).T
    ksin = np.asarray(sin, dtype=f).reshape(L, HD).T
    trig = np.ascontiguousarray(np.stack([qcos, qsin, kcos, ksin], axis=1))
    rotm = np.zeros((HD, HD), dtype=f)   # rot(q) = R @ q; pass R.T as lhsT
    half = HD // 2
    rotm[np.arange(half), np.arange(half) + half] = -1.0
    rotm[np.arange(half) + half, np.arange(half)] = 1.0
    idrot = np.ascontiguousarray(
        np.stack([np.eye(HD, dtype=f), rotm.T], axis=1)).astype(b16)

    maps = []
    for c in range(NCORES):
        kvh = c // 2
        wqT = np.zeros((D, SLOTS * HD), dtype=f)
        biases = np.zeros((HD, 6), dtype=f)
        for s in range(SLOTS):
            h = _head_of(c, s)
            if h is None:
                continue
            wqT[:, s * HD:(s + 1) * HD] = wq[h * HD:(h + 1) * HD, :].T
            biases[:, s] = bq[h * HD:(h + 1) * HD]
        biases[:, 4] = bk[kvh * HD:(kvh + 1) * HD]
        biases[:, 5] = bv[kvh * HD:(kvh + 1) * HD]
        woT = np.empty((H * HD, OSH), dtype=f)
        rows = slice(OSH * c, OSH * (c + 1))
        for gi, (jj, cc) in enumerate(REAL_JC):
            h = _head_of(cc, jj)
            woT[gi * HD:(gi + 1) * HD, :] = wo[rows, h * HD:(h + 1) * HD].T
        maps.append({
            "x": x,
            "wqT": wqT.astype(b16),
            "wkT": np.ascontiguousarray(wk[kvh * HD:(kvh + 1) * HD, :].T).astype(b16),
            "wvT": np.ascontiguousarray(wv[kvh * HD:(kvh + 1) * HD, :].T).astype(b16),
            "kcache": np.ascontiguousarray(key_cache[LI, kvh]).astype(b16),
            "vcache": np.ascontiguousarray(value_cache[LI, kvh]).astype(b16),
            "trig": trig,
            "biases": np.ascontiguousarray(biases),
            "idrot": idrot,
            "woT": woT.astype(b16),
        })
    return maps


def kernel(_trace=False, **inputs):
    from concourse.bass_utils import run_bass_kernel_spmd

    cp = int(np.asarray(inputs["cache_position"]))
    assert cp % 128 == 0 and 0 <= cp <= S_MAX - L, f"unsupported cache_position {cp}"

    maps = _shards(
        inputs["hidden_states"], inputs["cos"], inputs["sin"],
        inputs["cos_t"], inputs["sin_t"],
        inputs["key_cache"], inputs["value_cache"],
        inputs["wq"], inputs["bq"], inputs["wk"], inputs["bk"],
        inputs["wv"], inputs["bv"], inputs["wo"],
    )
    nc = _get_prog(cp)
    res = run_bass_kernel_spmd(nc, maps, core_ids=list(range(NCORES)),
                               trace=_trace)
    out = np.concatenate([r["out"] for r in res.results], axis=0)
    out = out.astype(np.float32).reshape(1, D, 1, L)
    if _trace:
        return out, res
    return out
